# revision 11
# baseline (speedup 1.0000x reference)
"""Trainium2 kernel for nn_ContrastiveLoss (N=4096, D=1024), SPMD over 8 NeuronCores.

Strategy (2x4-blocked similarity matrix, fp8 DoubleRow matmuls):
  - Host: l2-normalize back_VF/back_AF in f64, scale by 16 and quantize to
    e4m3, pre-transpose into DoubleRow-blocked layouts, compute diag sims
    and the pre-feature cosine term (both O(N*D), f64).
  - Cores form a 2x4 grid: core (r, c) computes the [2048, 1024] block
    E = exp(Vn[rows] @ An[cols]^T):
      * TensorE: 16 groups x 8 fp8 DoubleRow matmuls (K=256 each) into
        [128,1024] PSUM tiles, preceded by HAM-warmup matmuls sized to
        bridge the input-DMA window at full clock (8/8)
      * ScalarE: exp(PSUM / 256) -> bf16 SBUF tile, fused f32 row-sum
      * VectorE: bf16 column-sum accumulation across the 16 row chunks
      * last group computed in halves so the final add/DMA chain is short
      * DMA: deadline-ordered transfers on the two HWDGE queues (an k2-major
        on sync so chunks arrive in consumption order; vn mc-major on scalar
        in growing chunks)
    Outputs per core: rowsum [128, 17], bf16 column accumulator [128, 1024]
    (partition-folded on host).
  - Host: O(N) final assembly (log/ratio/sums) in f64.
"""

import os
import sys

import numpy as np

for _p in ("/opt/trn_rl_repo",):
    if _p not in sys.path and os.path.isdir(_p):
        sys.path.insert(0, _p)

N = 4096
D = 1024
NCORES = 8
RGRID = 2                # row groups
CGRID = 4                # col groups
RROWS = N // RGRID       # 2048 rows per core
CCOLS = N // CGRID       # 1024 cols per core
MCH = RROWS // 128       # 16 row chunks per core
KCH = D // 128           # 8 contraction chunks
KD2 = KCH // 2           # fp8 DoubleRow: contraction chunks of 256
NB = 512                 # matmul moving free dim
NBLK = CCOLS // NB       # 2 column blocks per core

MARGIN = 0.2
BALANCE = 0.5
BIAS = 1.0
EPS = 1e-18

FP8_SCALE = 16.0  # host pre-scale so e4m3 keeps the values out of subnormals

# HAM warmup: ~6 cold 512-col matmuls cover one 3.4us activity window
# (cold MM ~ (512+219)/1.2 ~ 610ns); extras keep PE busy until the input
# DMA lands (~12.4us) so the real stream starts at 8/8 clock with no
# idle window in between (idle >= 3.4us drops the clock to 4/8).
NWARM_BIG = 6
NWARM_EXTRA = 6

_CACHE = {}
LAST_RESULT = None  # BassKernelResults of the most recent run (for test harness)


def _build_nc():
    import concourse.bass as bass  # noqa: F401
    import concourse.bacc as bacc
    import concourse.tile as tile
    from concourse import mybir
    from contextlib import ExitStack

    BF16 = mybir.dt.bfloat16
    F32 = mybir.dt.float32
    FP8 = mybir.dt.float8e4
    Exp = mybir.ActivationFunctionType.Exp
    DoubleRow = mybir.MatmulPerfMode.DoubleRow

    nc = bacc.Bacc("TRN2", debug=False, num_devices=NCORES)

    # DRAM I/O (per core).
    # vnT[p, mc*1024 + k2*256 + i*128 + m] = Vn8[r*2048 + mc*128 + m,
    #                                            (2*k2+i)*128 + p]
    vnT_d = nc.dram_tensor("vnT", [128, MCH * KCH * 128], FP8, kind="ExternalInput")
    # anT[p, k2*2048 + b*1024 + i*512 + c] = An8[cg*1024 + b*512 + c,
    #                                            (2*k2+i)*128 + p]
    anT_d = nc.dram_tensor("anT", [128, KD2 * NBLK * 2 * NB], FP8, kind="ExternalInput")

    # rowsum[p, mc] = sum over this core's 1024 cols of E[mc*128 + p, :]
    # (the last row chunk is split in halves: cols 15 and 16)
    rowsum_d = nc.dram_tensor("rowsum", [128, MCH + 1], F32, kind="ExternalOutput")
    # esum[p, j] = sum over row chunks mc of E[mc*128 + p, j], bf16;
    # the 128-partition fold happens on host.
    esum_d = nc.dram_tensor("esum", [128, CCOLS], BF16, kind="ExternalOutput")

    with tile.TileContext(nc) as tc:
        with ExitStack() as ctx:
            singles = ctx.enter_context(tc.tile_pool(name="singles", bufs=1))

            vn_sb = singles.tile([128, MCH * KCH * 128], FP8, tag="vn")
            an_sb = singles.tile([128, KD2 * NBLK * 2 * NB], FP8, tag="an")

            # Deadline-ordered input transfers. The SDMA engines serve the
            # two HWDGE queues at roughly equal rates (~180 B/ns each), so
            # the T0-critical chunks are split across both queues and the
            # trailing chunks ride behind them in FIFO order.
            # sync (q1):   an k2={0,1} (512KB), vn row chunk 0 (128KB)
            # scalar (q10): an k2={2,3} (512KB), vn chunks 1-3, 4-7
            nc.sync.dma_start(an_sb[:, 0:4096], anT_d.ap()[:, 0:4096])
            nc.sync.dma_start(vn_sb[:, 0:1024], vnT_d.ap()[:, 0:1024])
            nc.scalar.dma_start(an_sb[:, 4096:8192], anT_d.ap()[:, 4096:8192])
            nc.scalar.dma_start(vn_sb[:, 1024:4096], vnT_d.ap()[:, 1024:4096])
            nc.scalar.dma_start(vn_sb[:, 4096:8192], vnT_d.ap()[:, 4096:8192])
            # vn row chunks 8-15 (1MB) are deadline-slack (needed ~T0+12us):
            # punt them to the schedule tail on the idle gpsimd SWDGE queue
            # so they don't steal SDMA bandwidth from the critical chunks.
            with tc.high_priority(offset=-1_000_000):
                nc.gpsimd.dma_start(
                    vn_sb[:, 8192:16384], vnT_d.ap()[:, 8192:16384]
                )

            efold16 = singles.tile([128, CCOLS], BF16, tag="efold16")
            rs = singles.tile([128, MCH + 1], F32, tag="rs")
            scr = singles.tile([128, 1], F32, tag="scr")
            ones_b = singles.tile([128, 1], BF16, tag="ones_b")
            nc.vector.memset(ones_b[:], 1.0)
            dummy = singles.tile([128, NB], BF16, tag="dummy")
            nc.vector.memset(dummy[:], 0.0)
            et15 = []
            for h in range(NBLK):
                et15_h = singles.tile([128, NB], BF16, tag=f"et15_{h}")
                et15.append(et15_h)

            psum = ctx.enter_context(tc.tile_pool(name="mm_psum", bufs=3, space="PSUM"))
            foldp = ctx.enter_context(tc.tile_pool(name="fold_psum", bufs=2, space="PSUM"))
            epool = ctx.enter_context(tc.tile_pool(name="etile", bufs=3))

            # HAM warmup: keep TensorE busy through the input-DMA window so
            # the clock gate is at 8/8 when the real matmul stream starts.
            wps = foldp.tile([128, NB], F32, tag="fold")
            nwarm = NWARM_BIG + NWARM_EXTRA
            for i in range(nwarm):
                nc.tensor.matmul(
                    wps[0:1, :], ones_b[:], dummy[:],
                    start=(i == 0), stop=(i == nwarm - 1),
                )

            # Main stream: 16 groups of 8 DoubleRow matmuls -> [128, 1024]
            # PSUM tile; ScalarE exp (bf16 out, f32 rowsum accum) drains it;
            # VectorE accumulates bf16 column sums across groups.
            descale = 1.0 / (FP8_SCALE * FP8_SCALE)
            for mc in range(MCH):
                ps = psum.tile([128, CCOLS], F32)
                for k2 in range(KD2):
                    w3 = (
                        vn_sb[:, mc * 1024 + k2 * 256 : mc * 1024 + (k2 + 1) * 256]
                        .rearrange("p (i m) -> p i m", i=2)
                    )
                    for b in range(NBLK):
                        a3 = (
                            an_sb[:, k2 * 2048 + b * 1024 : k2 * 2048 + (b + 1) * 1024]
                            .rearrange("p (i c) -> p i c", i=2)
                        )
                        nc.tensor.matmul(
                            ps[:, b * NB : (b + 1) * NB],
                            w3,
                            a3,
                            start=(k2 == 0),
                            stop=(k2 == KD2 - 1),
                            perf_mode=DoubleRow,
                        )
                if mc == 0:
                    nc.scalar.activation(
                        efold16[:], ps[:], Exp, scale=descale,
                        accum_out=rs[:, mc : mc + 1],
                    )
                elif mc < MCH - 1:
                    et = epool.tile([128, CCOLS], BF16)
                    nc.scalar.activation(
                        et[:], ps[:], Exp, scale=descale,
                        accum_out=rs[:, mc : mc + 1],
                    )
                    nc.vector.tensor_add(efold16[:], efold16[:], et[:])
                else:
                    # last group in halves: shortens the tail chain
                    # exp -> add -> esum DMA after the final matmul
                    for h in range(NBLK):
                        sl = slice(h * NB, (h + 1) * NB)
                        nc.scalar.activation(
                            et15[h][:], ps[:, sl], Exp, scale=descale,
                            accum_out=rs[:, mc + h : mc + h + 1],
                        )
                        nc.vector.tensor_add(
                            efold16[:, sl], efold16[:, sl], et15[h][:]
                        )
                        nc.sync.dma_start(esum_d.ap()[:, sl], efold16[:, sl])

            nc.scalar.dma_start(rowsum_d.ap(), rs[:])

    nc.compile()
    return nc


def _get_nc():
    if "nc" not in _CACHE:
        _CACHE["nc"] = _build_nc()
    return _CACHE["nc"]


def _prep_inputs(pre_VF, pre_AF, back_VF, back_AF):
    """Normalize + relayout on host; returns per-core in_maps and host terms."""
    import ml_dtypes

    V = np.asarray(back_VF, dtype=np.float64)
    A = np.asarray(back_AF, dtype=np.float64)
    Vn = V / np.sqrt((V * V).sum(-1, keepdims=True) + EPS)
    An = A / np.sqrt((A * A).sum(-1, keepdims=True) + EPS)
    diag = np.einsum("ij,ij->i", Vn, An)

    pv = np.asarray(pre_VF, dtype=np.float64)
    pa = np.asarray(pre_AF, dtype=np.float64)
    pre_cos = (pv * pa).sum(-1) / (
        np.sqrt((pv * pv).sum(-1) + EPS) * np.sqrt((pa * pa).sum(-1) + EPS)
    )

    fp8 = ml_dtypes.float8_e4m3
    Vn8 = (Vn * FP8_SCALE).astype(fp8)
    An8 = (An * FP8_SCALE).astype(fp8)

    # vnT[r][p, mc*1024 + k2*256 + i*128 + m] = Vn8[r*2048 + mc*128 + m,
    #                                                (2*k2+i)*128 + p]
    vnTs = [
        np.ascontiguousarray(
            Vn8[r * RROWS : (r + 1) * RROWS]
            .reshape(MCH, 128, KD2, 2, 128)
            .transpose(4, 0, 2, 3, 1)
            .reshape(128, MCH * KCH * 128)
        )
        for r in range(RGRID)
    ]
    # anT[c][p, k2*2048 + b*1024 + i*512 + cc] = An8[c*1024 + b*512 + cc,
    #                                                 (2*k2+i)*128 + p]
    anTs = [
        np.ascontiguousarray(
            An8[c * CCOLS : (c + 1) * CCOLS]
            .reshape(NBLK, NB, KD2, 2, 128)
            .transpose(4, 2, 0, 3, 1)
            .reshape(128, KD2 * NBLK * 2 * NB)
        )
        for c in range(CGRID)
    ]

    in_maps = []
    for core in range(NCORES):
        r, c = core // CGRID, core % CGRID
        in_maps.append({"vnT": vnTs[r], "anT": anTs[c]})
    return in_maps, diag, pre_cos


def _assemble(outs, diag, pre_cos):
    """O(N) final reduction on host, f64."""
    rowsum = np.zeros(N, dtype=np.float64)
    colsum = np.zeros(N, dtype=np.float64)
    for core in range(NCORES):
        r, c = core // CGRID, core % CGRID
        rsd = outs[core]["rowsum"].astype(np.float64)  # [128, MCH+1]
        rsd[:, MCH - 1] += rsd[:, MCH]
        rowsum[r * RROWS : (r + 1) * RROWS] += rsd[:, :MCH].T.reshape(RROWS)
        colsum[c * CCOLS : (c + 1) * CCOLS] += (
            outs[core]["esum"].astype(np.float64).sum(axis=0)
        )

    dE = np.exp(diag)
    pos = np.exp(diag - MARGIN)
    neg_V = rowsum - dE
    neg_A = colsum - dE
    L_V = np.log(pos / (pos + neg_V)).sum()
    L_A = np.log(pos / (pos + neg_A)).sum()
    L_pre = pre_cos.sum()

    loss = BALANCE * (-1.0 / BIAS) * (L_V + L_A) + (1.0 - BALANCE) * L_pre
    return np.array(loss, dtype=np.float32)


def kernel(pre_VF, pre_AF, back_VF, back_AF):
    global LAST_RESULT
    from concourse import bass_utils

    nc = _get_nc()
    in_maps, diag, pre_cos = _prep_inputs(pre_VF, pre_AF, back_VF, back_AF)
    res = bass_utils.run_bass_kernel_spmd(nc, in_maps, core_ids=list(range(NCORES)))
    LAST_RESULT = res
    return _assemble(res.results, diag, pre_cos)


# revision 15
# speedup vs baseline: 1.0730x; 1.0730x over previous
"""Trainium2 kernel for nn_ContrastiveLoss (N=4096, D=1024), SPMD over 8 NeuronCores.

Strategy (2x4-blocked similarity matrix, fp8 DoubleRow matmuls):
  - Host: l2-normalize back_VF/back_AF in f64, scale by 16 and quantize to
    e4m3, pre-transpose into DoubleRow-blocked layouts, compute diag sims
    and the pre-feature cosine term (both O(N*D), f64).
  - Cores form a 2x4 grid: core (r, c) computes the [2048, 1024] block
    E = exp(Vn[rows] @ An[cols]^T):
      * TensorE: 16 groups x 8 fp8 DoubleRow matmuls (K=256 each) into
        [128,1024] PSUM tiles, preceded by HAM-warmup matmuls sized to
        bridge the input-DMA window at full clock (8/8)
      * ScalarE: exp(PSUM / 256) -> bf16 SBUF tile, fused f32 row-sum
      * VectorE: bf16 column-sum accumulation across the 16 row chunks
      * last group computed in halves so the final add/DMA chain is short
      * DMA: deadline-ordered transfers on the two HWDGE queues (an k2-major
        on sync so chunks arrive in consumption order; vn mc-major on scalar
        in growing chunks)
    Outputs per core: rowsum [128, 17], bf16 column accumulator [128, 1024]
    (partition-folded on host).
  - Host: O(N) final assembly (log/ratio/sums) in f64.
"""

import os
import sys

import numpy as np

for _p in ("/opt/trn_rl_repo",):
    if _p not in sys.path and os.path.isdir(_p):
        sys.path.insert(0, _p)

N = 4096
D = 1024
NCORES = 8
RGRID = 2                # row groups
CGRID = 4                # col groups
RROWS = N // RGRID       # 2048 rows per core
CCOLS = N // CGRID       # 1024 cols per core
MCH = RROWS // 128       # 16 row chunks per core
KCH = D // 128           # 8 contraction chunks
KD2 = KCH // 2           # fp8 DoubleRow: contraction chunks of 256
NB = 512                 # matmul moving free dim
NBLK = CCOLS // NB       # 2 column blocks per core

MARGIN = 0.2
BALANCE = 0.5
BIAS = 1.0
EPS = 1e-18

FP8_SCALE = 16.0  # host pre-scale so e4m3 keeps the values out of subnormals

# HAM warmup: ~6 cold 512-col matmuls cover one 3.4us activity window
# (cold MM ~ (512+219)/1.2 ~ 610ns); extras keep PE busy until the input
# DMA lands (~12.4us) so the real stream starts at 8/8 clock with no
# idle window in between (idle >= 3.4us drops the clock to 4/8).
NWARM_BIG = 6
NWARM_EXTRA = 6

_CACHE = {}
LAST_RESULT = None  # BassKernelResults of the most recent run (for test harness)


def _build_nc():
    import concourse.bass as bass  # noqa: F401
    import concourse.bacc as bacc
    import concourse.tile as tile
    from concourse import mybir
    from contextlib import ExitStack

    BF16 = mybir.dt.bfloat16
    F32 = mybir.dt.float32
    FP8 = mybir.dt.float8e4
    Exp = mybir.ActivationFunctionType.Exp
    DoubleRow = mybir.MatmulPerfMode.DoubleRow

    nc = bacc.Bacc("TRN2", debug=False, num_devices=NCORES)

    # DRAM I/O (per core).
    # vnT[p, mc*1024 + k2*256 + i*128 + m] = Vn8[r*2048 + mc*128 + m,
    #                                            (2*k2+i)*128 + p]
    vnT_d = nc.dram_tensor("vnT", [128, MCH * KCH * 128], FP8, kind="ExternalInput")
    # anT[p, k2*2048 + b*1024 + i*512 + c] = An8[cg*1024 + b*512 + c,
    #                                            (2*k2+i)*128 + p]
    anT_d = nc.dram_tensor("anT", [128, KD2 * NBLK * 2 * NB], FP8, kind="ExternalInput")

    # rowsum[p, mc] = sum over this core's 1024 cols of E[mc*128 + p, :]
    # (the last row chunk is split in halves: cols 15 and 16)
    rowsum_d = nc.dram_tensor("rowsum", [128, MCH + 1], F32, kind="ExternalOutput")
    # esum[p, j] = sum over row chunks mc of E[mc*128 + p, j], bf16;
    # the 128-partition fold happens on host.
    esum_d = nc.dram_tensor("esum", [128, CCOLS], BF16, kind="ExternalOutput")

    with tile.TileContext(nc) as tc:
        with ExitStack() as ctx:
            singles = ctx.enter_context(tc.tile_pool(name="singles", bufs=1))

            vn_sb = singles.tile([128, MCH * KCH * 128], FP8, tag="vn")
            an_sb = singles.tile([128, KD2 * NBLK * 2 * NB], FP8, tag="an")

            # Deadline-ordered input transfers. The SDMA engines serve the
            # active queues at roughly equal rates (~175 B/ns each), so the
            # T0-critical chunks are split across the two HWDGE queues and
            # the trailing chunks ride behind them in FIFO order.
            # sync (q1):   vn row chunk 0 (128KB), an k2={0,1} (512KB)
            # scalar (q10): an k2={2,3} (512KB), vn chunks 1-3, 4-7
            nc.sync.dma_start(vn_sb[:, 0:1024], vnT_d.ap()[:, 0:1024])
            nc.sync.dma_start(an_sb[:, 0:4096], anT_d.ap()[:, 0:4096])
            nc.scalar.dma_start(an_sb[:, 4096:8192], anT_d.ap()[:, 4096:8192])
            nc.scalar.dma_start(vn_sb[:, 1024:4096], vnT_d.ap()[:, 1024:4096])
            nc.scalar.dma_start(vn_sb[:, 4096:8192], vnT_d.ap()[:, 4096:8192])

            efold16 = singles.tile([128, CCOLS], BF16, tag="efold16")
            rs = singles.tile([128, MCH + 1], F32, tag="rs")
            scr = singles.tile([128, 1], F32, tag="scr")
            ones_b = singles.tile([128, 1], BF16, tag="ones_b")
            nc.vector.memset(ones_b[:], 1.0)
            dummy = singles.tile([128, NB], BF16, tag="dummy")
            nc.vector.memset(dummy[:], 0.0)
            dummy8 = singles.tile([128, 128], FP8, tag="dummy8")
            nc.vector.memset(dummy8[:], 0.0)
            et15 = []
            for h in range(NBLK):
                et15_h = singles.tile([128, NB], BF16, tag=f"et15_{h}")
                et15.append(et15_h)

            psum = ctx.enter_context(tc.tile_pool(name="mm_psum", bufs=3, space="PSUM"))
            foldp = ctx.enter_context(tc.tile_pool(name="fold_psum", bufs=2, space="PSUM"))
            epool = ctx.enter_context(tc.tile_pool(name="etile", bufs=3))

            # HAM warmup: keep TensorE busy through the input-DMA window so
            # the clock gate is at 8/8 when the real matmul stream starts.
            wps = foldp.tile([128, NB], F32, tag="fold")
            nwarm = NWARM_BIG + NWARM_EXTRA - 1
            for i in range(nwarm):
                nc.tensor.matmul(
                    wps[0:1, :], ones_b[:], dummy[:],
                    start=(i == 0), stop=(i == nwarm - 1),
                )
            # Final warmup matmul doubles as the release gate for the vn
            # bulk DMA: it reads (as weights) the head of the region the
            # DMA writes, so the transfer cannot enter the SDMA pipe before
            # warmup ends -- keeping the early pipe free for the T0-critical
            # chunks. Reads garbage; the product is never consumed.
            wgate = (
                vn_sb[:, 8192:8448].rearrange("p (i m) -> p i m", i=2)
            )
            agate = dummy8[:].rearrange("p (i c) -> p i c", i=2)
            wps2 = foldp.tile([128, NB], F32, tag="fold")
            nc.tensor.matmul(
                wps2[:, 0:64], wgate, agate, start=True, stop=True,
                perf_mode=mybir.MatmulPerfMode.DoubleRow,
            )
            # vn row chunks 8-15 (1MB) are deadline-slack (needed ~T0+12us):
            # released by the gate matmul above, on the idle gpsimd SWDGE
            # queue.
            nc.gpsimd.dma_start(vn_sb[:, 8192:16384], vnT_d.ap()[:, 8192:16384])

            # Main stream: 16 groups of 8 DoubleRow matmuls -> [128, 1024]
            # PSUM tile; ScalarE exp (bf16 out, f32 rowsum accum) drains it;
            # VectorE accumulates bf16 column sums across groups.
            descale = 1.0 / (FP8_SCALE * FP8_SCALE)
            for mc in range(MCH):
                ps = psum.tile([128, CCOLS], F32)
                for k2 in range(KD2):
                    w3 = (
                        vn_sb[:, mc * 1024 + k2 * 256 : mc * 1024 + (k2 + 1) * 256]
                        .rearrange("p (i m) -> p i m", i=2)
                    )
                    for b in range(NBLK):
                        a3 = (
                            an_sb[:, k2 * 2048 + b * 1024 : k2 * 2048 + (b + 1) * 1024]
                            .rearrange("p (i c) -> p i c", i=2)
                        )
                        nc.tensor.matmul(
                            ps[:, b * NB : (b + 1) * NB],
                            w3,
                            a3,
                            start=(k2 == 0),
                            stop=(k2 == KD2 - 1),
                            perf_mode=DoubleRow,
                        )
                if mc == 0:
                    nc.scalar.activation(
                        efold16[:], ps[:], Exp, scale=descale,
                        accum_out=rs[:, mc : mc + 1],
                    )
                elif mc < MCH - 1:
                    et = epool.tile([128, CCOLS], BF16)
                    nc.scalar.activation(
                        et[:], ps[:], Exp, scale=descale,
                        accum_out=rs[:, mc : mc + 1],
                    )
                    nc.vector.tensor_add(efold16[:], efold16[:], et[:])
                else:
                    # last group in halves: shortens the tail chain
                    # exp -> add -> esum DMA after the final matmul
                    for h in range(NBLK):
                        sl = slice(h * NB, (h + 1) * NB)
                        nc.scalar.activation(
                            et15[h][:], ps[:, sl], Exp, scale=descale,
                            accum_out=rs[:, mc + h : mc + h + 1],
                        )
                        nc.vector.tensor_add(
                            efold16[:, sl], efold16[:, sl], et15[h][:]
                        )
                        nc.sync.dma_start(esum_d.ap()[:, sl], efold16[:, sl])

            nc.scalar.dma_start(rowsum_d.ap(), rs[:])

    nc.compile()
    return nc


def _get_nc():
    if "nc" not in _CACHE:
        _CACHE["nc"] = _build_nc()
    return _CACHE["nc"]


def _prep_inputs(pre_VF, pre_AF, back_VF, back_AF):
    """Normalize + relayout on host; returns per-core in_maps and host terms."""
    import ml_dtypes

    V = np.asarray(back_VF, dtype=np.float64)
    A = np.asarray(back_AF, dtype=np.float64)
    Vn = V / np.sqrt((V * V).sum(-1, keepdims=True) + EPS)
    An = A / np.sqrt((A * A).sum(-1, keepdims=True) + EPS)
    diag = np.einsum("ij,ij->i", Vn, An)

    pv = np.asarray(pre_VF, dtype=np.float64)
    pa = np.asarray(pre_AF, dtype=np.float64)
    pre_cos = (pv * pa).sum(-1) / (
        np.sqrt((pv * pv).sum(-1) + EPS) * np.sqrt((pa * pa).sum(-1) + EPS)
    )

    fp8 = ml_dtypes.float8_e4m3
    Vn8 = (Vn * FP8_SCALE).astype(fp8)
    An8 = (An * FP8_SCALE).astype(fp8)

    # vnT[r][p, mc*1024 + k2*256 + i*128 + m] = Vn8[r*2048 + mc*128 + m,
    #                                                (2*k2+i)*128 + p]
    vnTs = [
        np.ascontiguousarray(
            Vn8[r * RROWS : (r + 1) * RROWS]
            .reshape(MCH, 128, KD2, 2, 128)
            .transpose(4, 0, 2, 3, 1)
            .reshape(128, MCH * KCH * 128)
        )
        for r in range(RGRID)
    ]
    # anT[c][p, k2*2048 + b*1024 + i*512 + cc] = An8[c*1024 + b*512 + cc,
    #                                                 (2*k2+i)*128 + p]
    anTs = [
        np.ascontiguousarray(
            An8[c * CCOLS : (c + 1) * CCOLS]
            .reshape(NBLK, NB, KD2, 2, 128)
            .transpose(4, 2, 0, 3, 1)
            .reshape(128, KD2 * NBLK * 2 * NB)
        )
        for c in range(CGRID)
    ]

    in_maps = []
    for core in range(NCORES):
        r, c = core // CGRID, core % CGRID
        in_maps.append({"vnT": vnTs[r], "anT": anTs[c]})
    return in_maps, diag, pre_cos


def _assemble(outs, diag, pre_cos):
    """O(N) final reduction on host, f64."""
    rowsum = np.zeros(N, dtype=np.float64)
    colsum = np.zeros(N, dtype=np.float64)
    for core in range(NCORES):
        r, c = core // CGRID, core % CGRID
        rsd = outs[core]["rowsum"].astype(np.float64)  # [128, MCH+1]
        rsd[:, MCH - 1] += rsd[:, MCH]
        rowsum[r * RROWS : (r + 1) * RROWS] += rsd[:, :MCH].T.reshape(RROWS)
        colsum[c * CCOLS : (c + 1) * CCOLS] += (
            outs[core]["esum"].astype(np.float64).sum(axis=0)
        )

    dE = np.exp(diag)
    pos = np.exp(diag - MARGIN)
    neg_V = rowsum - dE
    neg_A = colsum - dE
    L_V = np.log(pos / (pos + neg_V)).sum()
    L_A = np.log(pos / (pos + neg_A)).sum()
    L_pre = pre_cos.sum()

    loss = BALANCE * (-1.0 / BIAS) * (L_V + L_A) + (1.0 - BALANCE) * L_pre
    return np.array(loss, dtype=np.float32)


def kernel(pre_VF, pre_AF, back_VF, back_AF):
    global LAST_RESULT
    from concourse import bass_utils

    nc = _get_nc()
    in_maps, diag, pre_cos = _prep_inputs(pre_VF, pre_AF, back_VF, back_AF)
    res = bass_utils.run_bass_kernel_spmd(nc, in_maps, core_ids=list(range(NCORES)))
    LAST_RESULT = res
    return _assemble(res.results, diag, pre_cos)


# revision 16
# speedup vs baseline: 1.1158x; 1.0398x over previous
"""Trainium2 kernel for nn_ContrastiveLoss (N=4096, D=1024), SPMD over 8 NeuronCores.

Strategy (2x4-blocked similarity matrix, fp8 DoubleRow matmuls):
  - Host: l2-normalize back_VF/back_AF in f64, scale by 16 and quantize to
    e4m3, pre-transpose into DoubleRow-blocked layouts, compute diag sims
    and the pre-feature cosine term (both O(N*D), f64).
  - Cores form a 2x4 grid: core (r, c) computes the [2048, 1024] block
    E = exp(Vn[rows] @ An[cols]^T):
      * TensorE: 16 groups x 8 fp8 DoubleRow matmuls (K=256 each) into
        [128,1024] PSUM tiles, preceded by HAM-warmup matmuls sized to
        bridge the input-DMA window at full clock (8/8)
      * ScalarE: exp(PSUM / 256) -> bf16 SBUF tile, fused f32 row-sum
      * VectorE: bf16 column-sum accumulation across the 16 row chunks
      * last group computed in halves so the final add/DMA chain is short
      * DMA: deadline-ordered transfers on the two HWDGE queues (an k2-major
        on sync so chunks arrive in consumption order; vn mc-major on scalar
        in growing chunks)
    Outputs per core: rowsum [128, 17], bf16 column accumulator [128, 1024]
    (partition-folded on host).
  - Host: O(N) final assembly (log/ratio/sums) in f64.
"""

import os
import sys

import numpy as np

for _p in ("/opt/trn_rl_repo",):
    if _p not in sys.path and os.path.isdir(_p):
        sys.path.insert(0, _p)

N = 4096
D = 1024
NCORES = 8
RGRID = 2                # row groups
CGRID = 4                # col groups
RROWS = N // RGRID       # 2048 rows per core
CCOLS = N // CGRID       # 1024 cols per core
MCH = RROWS // 128       # 16 row chunks per core
KCH = D // 128           # 8 contraction chunks
KD2 = KCH // 2           # fp8 DoubleRow: contraction chunks of 256
NB = 512                 # matmul moving free dim
NBLK = CCOLS // NB       # 2 column blocks per core

MARGIN = 0.2
BALANCE = 0.5
BIAS = 1.0
EPS = 1e-18

FP8_SCALE = 16.0  # host pre-scale so e4m3 keeps the values out of subnormals

# HAM warmup: ~6 cold 512-col matmuls cover one 3.4us activity window
# (cold MM ~ (512+219)/1.2 ~ 610ns); extras keep PE busy until the input
# DMA lands (~12.4us) so the real stream starts at 8/8 clock with no
# idle window in between (idle >= 3.4us drops the clock to 4/8).
NWARM_BIG = 6
NWARM_EXTRA = 6

_CACHE = {}
LAST_RESULT = None  # BassKernelResults of the most recent run (for test harness)


def _build_nc():
    import concourse.bass as bass  # noqa: F401
    import concourse.bacc as bacc
    import concourse.tile as tile
    from concourse import mybir
    from contextlib import ExitStack

    BF16 = mybir.dt.bfloat16
    F32 = mybir.dt.float32
    FP8 = mybir.dt.float8e4
    Exp = mybir.ActivationFunctionType.Exp
    DoubleRow = mybir.MatmulPerfMode.DoubleRow

    nc = bacc.Bacc("TRN2", debug=False, num_devices=NCORES)

    # DRAM I/O (per core).
    # vnT[p, mc*1024 + k2*256 + i*128 + m] = Vn8[r*2048 + mc*128 + m,
    #                                            (2*k2+i)*128 + p]
    vnT_d = nc.dram_tensor("vnT", [128, MCH * KCH * 128], FP8, kind="ExternalInput")
    # anT[p, k2*2048 + b*1024 + i*512 + c] = An8[cg*1024 + b*512 + c,
    #                                            (2*k2+i)*128 + p]
    anT_d = nc.dram_tensor("anT", [128, KD2 * NBLK * 2 * NB], FP8, kind="ExternalInput")

    # rowsum[p, mc] = sum over this core's 1024 cols of E[mc*128 + p, :]
    # (the last row chunk is split in halves: cols 15 and 16)
    rowsum_d = nc.dram_tensor("rowsum", [128, MCH + 1], F32, kind="ExternalOutput")
    # esum[p, j] = sum over row chunks mc of E[mc*128 + p, j], bf16;
    # the 128-partition fold happens on host.
    esum_d = nc.dram_tensor("esum", [128, CCOLS], BF16, kind="ExternalOutput")

    with tile.TileContext(nc) as tc:
        with ExitStack() as ctx:
            singles = ctx.enter_context(tc.tile_pool(name="singles", bufs=1))

            vn_sb = singles.tile([128, MCH * KCH * 128], FP8, tag="vn")
            an_sb = singles.tile([128, KD2 * NBLK * 2 * NB], FP8, tag="an")

            # Deadline-ordered input transfers. The SDMA engines serve the
            # active queues at roughly equal rates (~175 B/ns each), so the
            # T0-critical chunks are split across the two HWDGE queues and
            # the trailing chunks ride behind them in FIFO order.
            # sync (q1):   an k2={0,1} (512KB), vn row chunk 0 (128KB)
            # scalar (q10): an k2={2,3} (512KB), vn chunks 1-3, 4-7
            nc.sync.dma_start(an_sb[:, 0:4096], anT_d.ap()[:, 0:4096])
            nc.sync.dma_start(vn_sb[:, 0:1024], vnT_d.ap()[:, 0:1024])
            nc.scalar.dma_start(an_sb[:, 4096:8192], anT_d.ap()[:, 4096:8192])
            nc.scalar.dma_start(vn_sb[:, 1024:4096], vnT_d.ap()[:, 1024:4096])
            nc.scalar.dma_start(vn_sb[:, 4096:8192], vnT_d.ap()[:, 4096:8192])

            efold16 = singles.tile([128, CCOLS], BF16, tag="efold16")
            rs = singles.tile([128, MCH + 1], F32, tag="rs")
            scr = singles.tile([128, 1], F32, tag="scr")
            ones_b = singles.tile([128, 1], BF16, tag="ones_b")
            nc.vector.memset(ones_b[:], 1.0)
            dummy = singles.tile([128, NB], BF16, tag="dummy")
            nc.vector.memset(dummy[:], 0.0)
            dummy8 = singles.tile([128, 128], FP8, tag="dummy8")
            nc.vector.memset(dummy8[:], 0.0)
            et15 = []
            for h in range(NBLK):
                et15_h = singles.tile([128, NB], BF16, tag=f"et15_{h}")
                et15.append(et15_h)

            psum = ctx.enter_context(tc.tile_pool(name="mm_psum", bufs=3, space="PSUM"))
            foldp = ctx.enter_context(tc.tile_pool(name="fold_psum", bufs=2, space="PSUM"))
            epool = ctx.enter_context(tc.tile_pool(name="etile", bufs=3))

            # HAM warmup: keep TensorE busy through the input-DMA window so
            # the clock gate is at 8/8 when the real matmul stream starts.
            wps = foldp.tile([128, NB], F32, tag="fold")
            nwarm = NWARM_BIG + NWARM_EXTRA - 1
            for i in range(nwarm):
                nc.tensor.matmul(
                    wps[0:1, :], ones_b[:], dummy[:],
                    start=(i == 0), stop=(i == nwarm - 1),
                )
            # Final warmup matmul doubles as the release gate for the vn
            # bulk DMA: it reads (as weights) the head of the region the
            # DMA writes, so the transfer cannot enter the SDMA pipe before
            # warmup ends -- keeping the early pipe free for the T0-critical
            # chunks. Reads garbage; the product is never consumed.
            wgate = (
                vn_sb[:, 8192:8448].rearrange("p (i m) -> p i m", i=2)
            )
            agate = dummy8[:].rearrange("p (i c) -> p i c", i=2)
            wps2 = foldp.tile([128, NB], F32, tag="fold")
            nc.tensor.matmul(
                wps2[:, 0:64], wgate, agate, start=True, stop=True,
                perf_mode=mybir.MatmulPerfMode.DoubleRow,
            )
            # vn row chunks 8-15 (1MB) are deadline-slack (needed ~T0+12us):
            # released by the gate matmul above, on the idle gpsimd SWDGE
            # queue.
            nc.gpsimd.dma_start(vn_sb[:, 8192:16384], vnT_d.ap()[:, 8192:16384])

            # Main stream: 16 groups of 8 DoubleRow matmuls -> [128, 1024]
            # PSUM tile; ScalarE exp (bf16 out, f32 rowsum accum) drains it;
            # VectorE accumulates bf16 column sums across groups.
            descale = 1.0 / (FP8_SCALE * FP8_SCALE)
            for mc in range(MCH):
                ps = psum.tile([128, CCOLS], F32)
                for k2 in range(KD2):
                    w3 = (
                        vn_sb[:, mc * 1024 + k2 * 256 : mc * 1024 + (k2 + 1) * 256]
                        .rearrange("p (i m) -> p i m", i=2)
                    )
                    for b in range(NBLK):
                        a3 = (
                            an_sb[:, k2 * 2048 + b * 1024 : k2 * 2048 + (b + 1) * 1024]
                            .rearrange("p (i c) -> p i c", i=2)
                        )
                        nc.tensor.matmul(
                            ps[:, b * NB : (b + 1) * NB],
                            w3,
                            a3,
                            start=(k2 == 0),
                            stop=(k2 == KD2 - 1),
                            perf_mode=DoubleRow,
                        )
                if mc == 0:
                    nc.scalar.activation(
                        efold16[:], ps[:], Exp, scale=descale,
                        accum_out=rs[:, mc : mc + 1],
                    )
                elif mc < MCH - 1:
                    et = epool.tile([128, CCOLS], BF16)
                    nc.scalar.activation(
                        et[:], ps[:], Exp, scale=descale,
                        accum_out=rs[:, mc : mc + 1],
                    )
                    nc.vector.tensor_add(efold16[:], efold16[:], et[:])
                else:
                    # last group in halves: shortens the tail chain
                    # exp -> add -> esum DMA after the final matmul
                    for h in range(NBLK):
                        sl = slice(h * NB, (h + 1) * NB)
                        nc.scalar.activation(
                            et15[h][:], ps[:, sl], Exp, scale=descale,
                            accum_out=rs[:, mc + h : mc + h + 1],
                        )
                        nc.vector.tensor_add(
                            efold16[:, sl], efold16[:, sl], et15[h][:]
                        )
                        nc.sync.dma_start(esum_d.ap()[:, sl], efold16[:, sl])

            nc.scalar.dma_start(rowsum_d.ap(), rs[:])

    nc.compile()
    return nc


def _get_nc():
    if "nc" not in _CACHE:
        _CACHE["nc"] = _build_nc()
    return _CACHE["nc"]


def _prep_inputs(pre_VF, pre_AF, back_VF, back_AF):
    """Normalize + relayout on host; returns per-core in_maps and host terms."""
    import ml_dtypes

    V = np.asarray(back_VF, dtype=np.float64)
    A = np.asarray(back_AF, dtype=np.float64)
    Vn = V / np.sqrt((V * V).sum(-1, keepdims=True) + EPS)
    An = A / np.sqrt((A * A).sum(-1, keepdims=True) + EPS)
    diag = np.einsum("ij,ij->i", Vn, An)

    pv = np.asarray(pre_VF, dtype=np.float64)
    pa = np.asarray(pre_AF, dtype=np.float64)
    pre_cos = (pv * pa).sum(-1) / (
        np.sqrt((pv * pv).sum(-1) + EPS) * np.sqrt((pa * pa).sum(-1) + EPS)
    )

    fp8 = ml_dtypes.float8_e4m3
    Vn8 = (Vn * FP8_SCALE).astype(fp8)
    An8 = (An * FP8_SCALE).astype(fp8)

    # vnT[r][p, mc*1024 + k2*256 + i*128 + m] = Vn8[r*2048 + mc*128 + m,
    #                                                (2*k2+i)*128 + p]
    vnTs = [
        np.ascontiguousarray(
            Vn8[r * RROWS : (r + 1) * RROWS]
            .reshape(MCH, 128, KD2, 2, 128)
            .transpose(4, 0, 2, 3, 1)
            .reshape(128, MCH * KCH * 128)
        )
        for r in range(RGRID)
    ]
    # anT[c][p, k2*2048 + b*1024 + i*512 + cc] = An8[c*1024 + b*512 + cc,
    #                                                 (2*k2+i)*128 + p]
    anTs = [
        np.ascontiguousarray(
            An8[c * CCOLS : (c + 1) * CCOLS]
            .reshape(NBLK, NB, KD2, 2, 128)
            .transpose(4, 2, 0, 3, 1)
            .reshape(128, KD2 * NBLK * 2 * NB)
        )
        for c in range(CGRID)
    ]

    in_maps = []
    for core in range(NCORES):
        r, c = core // CGRID, core % CGRID
        in_maps.append({"vnT": vnTs[r], "anT": anTs[c]})
    return in_maps, diag, pre_cos


def _assemble(outs, diag, pre_cos):
    """O(N) final reduction on host, f64."""
    rowsum = np.zeros(N, dtype=np.float64)
    colsum = np.zeros(N, dtype=np.float64)
    for core in range(NCORES):
        r, c = core // CGRID, core % CGRID
        rsd = outs[core]["rowsum"].astype(np.float64)  # [128, MCH+1]
        rsd[:, MCH - 1] += rsd[:, MCH]
        rowsum[r * RROWS : (r + 1) * RROWS] += rsd[:, :MCH].T.reshape(RROWS)
        colsum[c * CCOLS : (c + 1) * CCOLS] += (
            outs[core]["esum"].astype(np.float64).sum(axis=0)
        )

    dE = np.exp(diag)
    pos = np.exp(diag - MARGIN)
    neg_V = rowsum - dE
    neg_A = colsum - dE
    L_V = np.log(pos / (pos + neg_V)).sum()
    L_A = np.log(pos / (pos + neg_A)).sum()
    L_pre = pre_cos.sum()

    loss = BALANCE * (-1.0 / BIAS) * (L_V + L_A) + (1.0 - BALANCE) * L_pre
    return np.array(loss, dtype=np.float32)


def kernel(pre_VF, pre_AF, back_VF, back_AF):
    global LAST_RESULT
    from concourse import bass_utils

    nc = _get_nc()
    in_maps, diag, pre_cos = _prep_inputs(pre_VF, pre_AF, back_VF, back_AF)
    res = bass_utils.run_bass_kernel_spmd(nc, in_maps, core_ids=list(range(NCORES)))
    LAST_RESULT = res
    return _assemble(res.results, diag, pre_cos)
